# revision 13
# baseline (speedup 1.0000x reference)
"""Trainium2 Bass kernel for nn_EnergyTransformerEncoderLayer_51402168598868.

Strategy: data-parallel over batch B=16 across 8 NeuronCores (2 batches per
core).  Each core also receives batch 2c+2 (wrapped) so the KL term's
(aw[b], self_attn[b+1]) pairs can be computed entirely locally; the host sums
the 15 valid pair contributions.  All activations use "T-space" layouts
(feature dim on partitions) so every matmul contracts over partitions; the
host pre-transposes x and the weights, and transposes the output back.
Matmuls run as float32r (full PE rate, ~1e-4 rel rounding).
"""

import os
import sys
import types

if "/opt/trn_rl_repo" not in sys.path:
    sys.path.insert(0, "/opt/trn_rl_repo")

import numpy as np

B, S, D, F = 16, 1024, 1024, 4096
NCORE = 8
EPS = 1e-6
LN_EPS = 1e-5
P = 128
DC = D // P          # 8 feature chunks
SC = S // P          # 8 sequence chunks
FC = F // P          # 32 ffn chunks
H = 512              # matmul free-dim tile (one PSUM bank of f32)
NH = S // H          # 2 halves

_CACHE = {}


def _install_ntff_hook():
    """Optional: register the NTFF profiling hook (for HW exec timing)."""
    try:
        import antenv
        if getattr(antenv, "axon_hooks", None) is not None:
            return
        hooks = types.ModuleType("antenv.axon_hooks")
        hooks._hook = None

        def set_axon_ntff_profile_hook(h):
            hooks._hook = h

        def get_axon_ntff_profile_hook():
            return hooks._hook

        hooks.set_axon_ntff_profile_hook = set_axon_ntff_profile_hook
        hooks.get_axon_ntff_profile_hook = get_axon_ntff_profile_hook
        antenv.axon_hooks = hooks
        sys.modules["antenv.axon_hooks"] = hooks
        from trn_agent_boot.trn_boot import _ntff_profile_via_ctypes
        set_axon_ntff_profile_hook(
            _ntff_profile_via_ctypes("/opt/axon/libaxon_pjrt.so"))
    except Exception:
        pass


def _build():
    import concourse.mybir as mybir
    import concourse.tile as tile
    from concourse import bacc

    F32 = mybir.dt.float32
    F32R = mybir.dt.float32r
    AF = mybir.ActivationFunctionType
    ALU = mybir.AluOpType
    AX = mybir.AxisListType

    nc = bacc.Bacc(None, target_bir_lowering=False)

    # ---- DRAM parameters (per-core) ----
    xT = nc.declare_dram_parameter("xT", [3, D, S], F32R, isOutput=False)
    WqT = nc.declare_dram_parameter("WqT", [D, D], F32R, isOutput=False)
    WkT = nc.declare_dram_parameter("WkT", [D, D], F32R, isOutput=False)
    WvT = nc.declare_dram_parameter("WvT", [D, D], F32R, isOutput=False)
    WoT = nc.declare_dram_parameter("WoT", [D, D], F32R, isOutput=False)
    W1T = nc.declare_dram_parameter("W1T", [D, F], F32R, isOutput=False)
    W2T = nc.declare_dram_parameter("W2T", [F, D], F32R, isOutput=False)
    bq32 = nc.declare_dram_parameter("bq32", [D], F32, isOutput=False)
    bkv = nc.declare_dram_parameter("bk", [D], F32, isOutput=False)
    boe = nc.declare_dram_parameter("bo_eff", [D], F32, isOutput=False)
    b1v = nc.declare_dram_parameter("b1", [F], F32, isOutput=False)
    b2v = nc.declare_dram_parameter("b2", [D], F32, isOutput=False)
    g1v = nc.declare_dram_parameter("g1", [D], F32, isOutput=False)
    be1v = nc.declare_dram_parameter("be1", [D], F32, isOutput=False)
    g2v = nc.declare_dram_parameter("g2", [D], F32, isOutput=False)
    be2v = nc.declare_dram_parameter("be2", [D], F32, isOutput=False)
    outT = nc.declare_dram_parameter("outT", [2, D, S], F32, isOutput=True)
    klp = nc.declare_dram_parameter("klp", [1, 2], F32, isOutput=True)

    with tile.TileContext(nc) as tc:
        arena = tc.alloc_tile_pool(name="arena", bufs=4)
        bcp = tc.alloc_tile_pool(name="bcp", bufs=2)
        chk = tc.alloc_tile_pool(name="chk", bufs=4)
        wst = tc.alloc_tile_pool(name="wst", bufs=2)
        rows = tc.alloc_tile_pool(name="rows", bufs=3)
        tiny = tc.alloc_tile_pool(name="tiny", bufs=8)
        sing = tc.alloc_tile_pool(name="sing", bufs=1)
        psmm = tc.alloc_tile_pool(name="psmm", bufs=6, space="PSUM")
        psrow = tc.alloc_tile_pool(name="psrow", bufs=2, space="PSUM")
        dram = tc.alloc_tile_pool(name="dram", bufs=1, space="DRAM")

        def big(dtype=F32R, shape=None):
            return arena.tile(shape or [P, DC, S], dtype, tag="big", name="big")

        # ---- constants / persistent small tiles ----
        def vec_pc(name, dram_ap, n_chunks):
            t = sing.tile([P, n_chunks], F32, tag=name, name=name)
            nc.sync.dma_start(out=t, in_=dram_ap.rearrange("(c p) -> p c", p=P))
            return t

        bq_t = vec_pc("bq", bq32[:], DC)
        bk_t = vec_pc("bk", bkv[:], DC)
        bo_t = vec_pc("bo", boe[:], DC)
        b2_t = vec_pc("b2", b2v[:], DC)
        g1_t = vec_pc("g1", g1v[:], DC)
        be1_t = vec_pc("be1", be1v[:], DC)
        g2_t = vec_pc("g2", g2v[:], DC)
        be2_t = vec_pc("be2", be2v[:], DC)
        b1_t = vec_pc("b1", b1v[:], FC)

        eps_t = sing.tile([P, 1], F32, tag="eps")
        nc.vector.memset(eps_t, EPS)
        lneps11 = sing.tile([1, 1], F32, tag="lneps")
        nc.vector.memset(lneps11, LN_EPS)
        ones_f = sing.tile([P, 1], F32, tag="onesf")
        nc.vector.memset(ones_f, 1.0)
        ones_r = sing.tile([P, 1], F32R, tag="onesr")
        nc.vector.tensor_copy(out=ones_r, in_=ones_f)

        nm_t = [sing.tile([P, SC], F32, tag=f"nm{i}", name=f"nm{i}") for i in range(2)]
        rinv_t = [sing.tile([P, SC], F32, tag=f"rinv{i}", name=f"rinv{i}") for i in range(2)]
        kl_cols = sing.tile([P, SC], F32, tag="klcols")
        spill = [dram.tile([P, SC, S], F32, tag=f"sp{i}", name=f"sp{i}") for i in range(2)]

        # ================= helpers =================

        def proj(dst, w_dram, src, bias_pc, scale, evict):
            """dst[o, s] = evict(sum_k W[k, o].T @ src[k, s]); weights
            streamed as [P, DC, P] column blocks."""
            for oc in range(DC):
                wcol = wst.tile([P, DC, P], F32R, tag="wcol", name="wcol")
                nc.sync.dma_start(
                    out=wcol,
                    in_=w_dram[:, oc * P:(oc + 1) * P].rearrange(
                        "(c p) i -> p c i", p=P))
                for h in range(NH):
                    ps = psmm.tile([P, H], F32, tag="ps", name="ps")
                    for kc in range(DC):
                        nc.tensor.matmul(
                            ps,
                            wcol[:, kc, :],
                            src[:, kc, h * H:(h + 1) * H],
                            start=(kc == 0), stop=(kc == DC - 1))
                    evict(dst, oc, h, ps, bias_pc, scale)

        def evict_act(dst, oc, h, ps, bias_pc, scale):
            nc.scalar.activation(
                out=dst[:, oc, h * H:(h + 1) * H], in_=ps, func=AF.Identity,
                bias=bias_pc[:, oc:oc + 1], scale=scale)

        def evict_copy(dst, oc, h, ps, bias_pc, scale):
            nc.vector.tensor_copy(out=dst[:, oc, h * H:(h + 1) * H], in_=ps)

        def softmax_rows(score_mms, nm_col, rinv_col, aw_chunk):
            """score_mms: emits the two psum halves; computes stats and
            evicts exp(score - max) into aw_chunk [P, S], rowsums -> rinv."""
            ps_a, ps_b = score_mms()
            nma = tiny.tile([P, 1], F32, tag="t", name="t")
            nmb = tiny.tile([P, 1], F32, tag="t", name="t")
            nc.vector.tensor_reduce(out=nma, in_=ps_a, axis=AX.X, op=ALU.max,
                                    negate=True)
            nc.vector.tensor_reduce(out=nmb, in_=ps_b, axis=AX.X, op=ALU.max,
                                    negate=True)
            nc.vector.tensor_tensor(out=nm_col, in0=nma, in1=nmb, op=ALU.min)
            rsa = tiny.tile([P, 1], F32, tag="t", name="t")
            rsb = tiny.tile([P, 1], F32, tag="t", name="t")
            nc.scalar.activation(out=aw_chunk[:, 0:H], in_=ps_a, func=AF.Exp,
                                 bias=nm_col, scale=1.0, accum_out=rsa)
            nc.scalar.activation(out=aw_chunk[:, H:S], in_=ps_b, func=AF.Exp,
                                 bias=nm_col, scale=1.0, accum_out=rsb)
            rs = tiny.tile([P, 1], F32, tag="t", name="t")
            nc.vector.tensor_tensor(out=rs, in0=rsa, in1=rsb, op=ALU.add)
            nc.vector.reciprocal(out=rinv_col, in_=rs)

        def layer_norm_T(x_tile, g_pc, be_pc):
            """In-place LayerNorm over the feature (partition x chunk) dim of
            x_tile [P, DC, S] (f32r):  x <- (x - mu) * rstd * g  (+ be via ACT
            if be_pc given).  2 DVE passes:  t = (x*g)*rstd_b ;
            x = (negmu_rstd_b*g) + t."""
            xf = x_tile.bitcast(F32)
            nmu_row = rows.tile([1, S], F32, tag="row", name="row")
            m2_row = rows.tile([1, S], F32, tag="row", name="row")
            for h in range(NH):
                pr = psrow.tile([1, H], F32, tag="pr", name="pr")
                for c in range(DC):
                    nc.tensor.matmul(pr, ones_r,
                                     x_tile[:, c, h * H:(h + 1) * H],
                                     start=(c == 0), stop=(c == DC - 1))
                nc.scalar.activation(out=nmu_row[0:1, h * H:(h + 1) * H],
                                     in_=pr, func=AF.Copy, scale=-1.0 / D)
                pr2 = psrow.tile([1, H], F32, tag="pr", name="pr")
                for c in range(DC):
                    sq = chk.tile([P, H], F32R, tag="c", name="c")
                    nc.vector.tensor_tensor(
                        out=sq, in0=xf[:, c, h * H:(h + 1) * H],
                        in1=xf[:, c, h * H:(h + 1) * H], op=ALU.mult)
                    nc.tensor.matmul(pr2, ones_r, sq,
                                     start=(c == 0), stop=(c == DC - 1))
                nc.scalar.activation(out=m2_row[0:1, h * H:(h + 1) * H],
                                     in_=pr2, func=AF.Copy, scale=1.0 / D)
            musq = rows.tile([1, S], F32, tag="row", name="row")
            nc.vector.tensor_tensor(out=musq, in0=nmu_row, in1=nmu_row,
                                    op=ALU.mult)
            nc.vector.tensor_tensor(out=m2_row, in0=m2_row, in1=musq,
                                    op=ALU.subtract)
            nc.scalar.activation(out=m2_row, in_=m2_row, func=AF.Sqrt,
                                 bias=lneps11, scale=1.0)
            nc.vector.reciprocal_approx_fast(out=m2_row, in_=m2_row)  # rstd
            nc.vector.tensor_tensor(out=nmu_row, in0=nmu_row, in1=m2_row,
                                    op=ALU.mult)  # -mu * rstd
            nmur_b = bcp.tile([P, S], F32, tag="bc", name="bc")
            rstd_b = bcp.tile([P, S], F32, tag="bc", name="bc")
            nc.gpsimd.partition_broadcast(nmur_b, nmu_row[0:1, :])
            nc.gpsimd.partition_broadcast(rstd_b, m2_row[0:1, :])
            for c in range(DC):
                t = chk.tile([P, S], F32, tag="c", name="c")
                nc.vector.scalar_tensor_tensor(
                    out=t, in0=xf[:, c, :], scalar=g_pc[:, c:c + 1],
                    in1=rstd_b, op0=ALU.mult, op1=ALU.mult)
                nc.vector.scalar_tensor_tensor(
                    out=x_tile[:, c, :], in0=nmur_b,
                    scalar=g_pc[:, c:c + 1], in1=t,
                    op0=ALU.mult, op1=ALU.add)
                if be_pc is not None:
                    nc.scalar.activation(
                        out=x_tile[:, c, :], in_=x_tile[:, c, :],
                        func=AF.Identity, bias=be_pc[:, c:c + 1], scale=1.0)

        def kl_pair(pair_idx, a_slot, xb_tile):
            """KL contribution of (aw[a_slot] from spill, sa from xb_tile)."""
            for sc_i in range(SC):
                def sa_mms(sc_i=sc_i):
                    out = []
                    for h in range(NH):
                        ps = psmm.tile([P, H], F32, tag="ps", name="ps")
                        for kc in range(DC):
                            nc.tensor.matmul(
                                ps,
                                xb_tile[:, kc, sc_i * P:(sc_i + 1) * P],
                                xb_tile[:, kc, h * H:(h + 1) * H],
                                start=(kc == 0), stop=(kc == DC - 1))
                        out.append(ps)
                    return out
                e_sa = chk.tile([P, S], F32, tag="c", name="c")
                nm_sa = tiny.tile([P, 1], F32, tag="t", name="t")
                rinv_sa = tiny.tile([P, 1], F32, tag="t", name="t")
                softmax_rows(sa_mms, nm_sa, rinv_sa, e_sa)
                # rcp = 1 / (sa + EPS), in place over e_sa
                nc.vector.tensor_scalar(out=e_sa, in0=e_sa, scalar1=rinv_sa,
                                        scalar2=EPS, op0=ALU.mult, op1=ALU.add)
                nc.vector.reciprocal_approx_fast(out=e_sa, in_=e_sa)
                aw_c = chk.tile([P, S], F32, tag="c", name="c")
                nc.sync.dma_start(out=aw_c, in_=spill[a_slot][:, sc_i, :])
                klq = chk.tile([P, S], F32, tag="c", name="c")
                nc.vector.scalar_tensor_tensor(
                    out=klq, in0=aw_c, scalar=rinv_t[a_slot][:, sc_i:sc_i + 1],
                    in1=e_sa, op0=ALU.mult, op1=ALU.mult)
                nc.scalar.activation(out=klq, in_=klq, func=AF.Ln,
                                     bias=eps_t, scale=1.0)
                klr = tiny.tile([P, 1], F32, tag="t", name="t")
                nc.vector.scalar_tensor_tensor(
                    out=klq, in0=aw_c, scalar=rinv_t[a_slot][:, sc_i:sc_i + 1],
                    in1=klq, op0=ALU.mult, op1=ALU.mult, accum_out=klr)
                nc.vector.tensor_copy(out=kl_cols[:, sc_i:sc_i + 1], in_=klr)
            # reduce kl_cols over partitions and chunks -> klp[0, pair_idx]
            prk = psrow.tile([1, SC], F32, tag="pr", name="pr")
            nc.tensor.matmul(prk, ones_f.bitcast(F32), kl_cols,
                             start=True, stop=True)
            kl11 = tiny.tile([1, 1], F32, tag="t", name="t")
            nc.vector.tensor_reduce(out=kl11, in_=prk, axis=AX.X, op=ALU.add)
            klsb = tiny.tile([1, 1], F32, tag="t", name="t")
            nc.vector.tensor_copy(out=klsb, in_=kl11)
            nc.sync.dma_start(out=klp[0:1, pair_idx:pair_idx + 1], in_=klsb)

        # ================= main per-slot loop =================
        for slot in range(2):
            # --- load xT for this slot ---
            xTs = big()
            nc.sync.dma_start(
                out=xTs, in_=xT[slot].rearrange("(c p) s -> p c s", p=P))
            xfs = xTs.bitcast(F32)

            # --- Q/K projections (scaled by 1/32 on Q) ---
            qT = big()
            proj(qT, WqT, xTs, bq_t, 1.0 / 32.0, evict_act)
            kT = big()
            proj(kT, WkT, xTs, bk_t, 1.0, evict_act)

            # --- scores softmax (S-layout) -> aw spill + stats ---
            for sc_i in range(SC):
                def sc_mms(sc_i=sc_i):
                    out = []
                    for h in range(NH):
                        ps = psmm.tile([P, H], F32, tag="ps", name="ps")
                        for kc in range(DC):
                            nc.tensor.matmul(
                                ps,
                                qT[:, kc, sc_i * P:(sc_i + 1) * P],
                                kT[:, kc, h * H:(h + 1) * H],
                                start=(kc == 0), stop=(kc == DC - 1))
                        out.append(ps)
                    return out
                aw_c = chk.tile([P, S], F32, tag="c", name="c")
                nm_sc = tiny.tile([P, 1], F32, tag="t", name="t")
                rinv_sc = tiny.tile([P, 1], F32, tag="t", name="t")
                softmax_rows(sc_mms, nm_sc, rinv_sc, aw_c)
                nc.vector.tensor_copy(out=nm_t[slot][:, sc_i:sc_i + 1],
                                      in_=nm_sc)
                nc.vector.tensor_copy(out=rinv_t[slot][:, sc_i:sc_i + 1],
                                      in_=rinv_sc)
                nc.sync.dma_start(out=spill[slot][:, sc_i, :], in_=aw_c)

            # --- awT = exp(scoresT), unnormalized (T-layout) ---
            awT = big()
            for tc_i in range(SC):
                for h in range(NH):
                    ps = psmm.tile([P, H], F32, tag="ps", name="ps")
                    for kc in range(DC):
                        nc.tensor.matmul(
                            ps,
                            kT[:, kc, tc_i * P:(tc_i + 1) * P],
                            qT[:, kc, h * H:(h + 1) * H],
                            start=(kc == 0), stop=(kc == DC - 1))
                    nc.scalar.activation(
                        out=awT[:, tc_i, h * H:(h + 1) * H], in_=ps,
                        func=AF.Exp, scale=1.0)

            # --- combo row: exp(-max)*rinv, broadcast over partitions ---
            combo_pc = tiny.tile([P, SC], F32, tag="combo", name="combo")
            nc.scalar.activation(out=combo_pc, in_=nm_t[slot], func=AF.Exp,
                                 scale=1.0)
            nc.vector.tensor_tensor(out=combo_pc, in0=combo_pc,
                                    in1=rinv_t[slot], op=ALU.mult)
            combo_d = dram.tile([S], F32, tag=f"combod{slot}", name="combod")
            nc.gpsimd.dma_start(
                out=combo_d.rearrange("(c p) -> p c", p=P), in_=combo_pc)
            combo_row = rows.tile([1, S], F32, tag="row", name="row")
            nc.gpsimd.dma_start(out=combo_row, in_=combo_d.rearrange("(o s) -> o s", o=1))
            combo_b = bcp.tile([P, S], F32, tag="bc", name="bc")
            nc.gpsimd.partition_broadcast(combo_b, combo_row[0:1, :])

            # --- V projection (normal [t, i] layout) ---
            # V.T[i, t] would need xT as rhs; instead compute V[t, i] with
            # WvT chunks as rhs, streamed [P, S] per kc (uses each chunk for
            # all tc so keep kc outer over psum pairs per tc: restructure
            # with tc outer and wv chunks cached in wst (8 x 4KB).
            vN = big()
            wv_chunks = []
            for kc in range(DC):
                wvk = wst.tile([P, S], F32R, tag=f"wv{kc}", name="wvk", bufs=1)
                nc.sync.dma_start(
                    out=wvk, in_=WvT[kc * P:(kc + 1) * P, :])
                wv_chunks.append(wvk)
            for tc_i in range(SC):
                for h in range(NH):
                    ps = psmm.tile([P, H], F32, tag="ps", name="ps")
                    for kc in range(DC):
                        nc.tensor.matmul(
                            ps,
                            xTs[:, kc, tc_i * P:(tc_i + 1) * P],
                            wv_chunks[kc][:, h * H:(h + 1) * H],
                            start=(kc == 0), stop=(kc == DC - 1))
                    nc.vector.tensor_copy(
                        out=vN[:, tc_i, h * H:(h + 1) * H], in_=ps)

            # --- attnvT[i, s] = sum_t V[t, i] * awT[t, s], scaled by combo ---
            avT = big()
            for ic in range(DC):
                for h in range(NH):
                    ps = psmm.tile([P, H], F32, tag="ps", name="ps")
                    for tc_i in range(SC):
                        nc.tensor.matmul(
                            ps,
                            vN[:, tc_i, ic * P:(ic + 1) * P],
                            awT[:, tc_i, h * H:(h + 1) * H],
                            start=(tc_i == 0), stop=(tc_i == SC - 1))
                    nc.vector.tensor_tensor(
                        out=avT[:, ic, h * H:(h + 1) * H], in0=ps,
                        in1=combo_b[:, h * H:(h + 1) * H], op=ALU.mult)

            # --- output projection + residual -> hpre (becomes hT) ---
            hT = big()
            for oc in range(DC):
                wocol = wst.tile([P, DC, P], F32R, tag="wcol", name="wcol")
                nc.sync.dma_start(
                    out=wocol,
                    in_=WoT[:, oc * P:(oc + 1) * P].rearrange(
                        "(c p) i -> p c i", p=P))
                for h in range(NH):
                    ps = psmm.tile([P, H], F32, tag="ps", name="ps")
                    for kc in range(DC):
                        nc.tensor.matmul(
                            ps,
                            wocol[:, kc, :],
                            avT[:, kc, h * H:(h + 1) * H],
                            start=(kc == 0), stop=(kc == DC - 1))
                    nc.vector.scalar_tensor_tensor(
                        out=hT[:, oc, h * H:(h + 1) * H], in0=ps,
                        scalar=bo_t[:, oc:oc + 1],
                        in1=xfs[:, oc, h * H:(h + 1) * H],
                        op0=ALU.add, op1=ALU.add)

            # --- KL pairs (only emitted in slot 1; needs sa of this slot) ---
            if slot == 1:
                kl_pair(0, 0, xTs)

            # --- LN1 in place: hT = LN(x + attn_out) ---
            layer_norm_T(hT, g1_t, None)

            if slot == 1:
                xT2 = big()
                nc.sync.dma_start(
                    out=xT2, in_=xT[2].rearrange("(c p) s -> p c s", p=P))
                kl_pair(1, 1, xT2)

            # --- FFN ---
            opre = big()
            opf = opre.bitcast(F32)
            hf = hT.bitcast(F32)
            for h in range(NH):
                for fh in range(2):
                    gT = big(shape=[P, FC // 2, H])
                    for fl in range(FC // 2):
                        fabs = fh * (FC // 2) + fl
                        w1f = wst.tile([P, DC, P], F32R, tag="wcol", name="w1f")
                        nc.sync.dma_start(
                            out=w1f,
                            in_=W1T[:, fabs * P:(fabs + 1) * P].rearrange(
                                "(c p) f -> p c f", p=P))
                        ps = psmm.tile([P, H], F32, tag="ps", name="ps")
                        for kc in range(DC):
                            nc.tensor.matmul(
                                ps, w1f[:, kc, :],
                                hT[:, kc, h * H:(h + 1) * H],
                                start=(kc == 0), stop=(kc == DC - 1))
                        nc.scalar.activation(
                            out=gT[:, fl, :], in_=ps, func=AF.Relu,
                            bias=b1_t[:, fabs:fabs + 1], scale=1.0)
                    for ic in range(DC):
                        ps = psmm.tile([P, H], F32, tag="ps", name="ps")
                        for fq in range(2):
                            w2q = wst.tile([P, DC, P], F32R, tag="wcol",
                                           name="w2q")
                            base = fh * (F // 2) + fq * (F // 4)
                            nc.sync.dma_start(
                                out=w2q,
                                in_=W2T[base:base + F // 4,
                                        ic * P:(ic + 1) * P].rearrange(
                                    "(fl p) i -> p fl i", p=P))
                            for fl in range(DC):
                                fg = fq * DC + fl
                                nc.tensor.matmul(
                                    ps, w2q[:, fl, :], gT[:, fg, :],
                                    start=(fg == 0),
                                    stop=(fg == FC // 2 - 1))
                        if fh == 0:
                            nc.vector.scalar_tensor_tensor(
                                out=opre[:, ic, h * H:(h + 1) * H], in0=ps,
                                scalar=b2_t[:, ic:ic + 1],
                                in1=hf[:, ic, h * H:(h + 1) * H],
                                op0=ALU.add, op1=ALU.add)
                        else:
                            nc.vector.tensor_tensor(
                                out=opre[:, ic, h * H:(h + 1) * H], in0=ps,
                                in1=opf[:, ic, h * H:(h + 1) * H], op=ALU.add)

            # --- LN2 in place -> final output, DMA out ---
            layer_norm_T(opre, g2_t, be2_t)
            for c in range(DC):
                nc.sync.dma_start(out=outT[slot, c * P:(c + 1) * P, :],
                                  in_=opf[:, c, :])

        for p in (dram, psrow, psmm, sing, tiny, rows, wst, chk, bcp, arena):
            p.release()

    nc.compile()
    return nc


def _get_program():
    if "nc" not in _CACHE:
        _CACHE["nc"] = _build()
    return _CACHE["nc"]


def kernel(x, Wq, bq, Wk, bk, Wv, bv, Wo, bo, g1, be1, W1, b1, W2, b2, g2, be2):
    from concourse.bass_utils import run_bass_kernel_spmd

    trace = os.environ.get("BASS_KERNEL_TRACE", "") == "1"
    if trace:
        _install_ntff_hook()

    f32 = np.float32
    x = np.asarray(x, f32)
    asf = lambda a: np.ascontiguousarray(np.asarray(a, f32))
    WqTn = asf(np.asarray(Wq, f32).T)
    WkTn = asf(np.asarray(Wk, f32).T)
    WvTn = asf(np.asarray(Wv, f32).T)
    WoTn = asf(np.asarray(Wo, f32).T)
    W1Tn = asf(np.asarray(W1, f32).T)
    W2Tn = asf(np.asarray(W2, f32).T)
    bq32n = asf(np.asarray(bq, f32) / 32.0)
    # attnv is computed without +bv; fold the exact linear correction into bo
    bo_eff = asf(np.asarray(bo, f32) + np.asarray(Wo, f32) @ np.asarray(bv, f32))
    xT_all = np.ascontiguousarray(x.transpose(0, 2, 1))

    # LN1's additive bias be1 folds exactly into the FFN biases:
    #   relu(h@W1.T + b1) with h = h' + be1  ->  b1_eff = b1 + W1 @ be1
    #   out-LN input (h + ff)               ->  b2_eff = b2 + be1
    b1_eff = asf(np.asarray(b1, f32) + np.asarray(W1, f32) @ np.asarray(be1, f32))
    b2_eff = asf(np.asarray(b2, f32) + np.asarray(be1, f32))
    shared = dict(
        WqT=WqTn, WkT=WkTn, WvT=WvTn, WoT=WoTn, W1T=W1Tn, W2T=W2Tn,
        bq32=bq32n, bk=asf(bk), bo_eff=bo_eff, b1=b1_eff, b2=b2_eff,
        g1=asf(g1), be1=asf(be1), g2=asf(g2), be2=asf(be2))

    in_maps = []
    for c in range(NCORE):
        sl = [2 * c, 2 * c + 1, (2 * c + 2) % B]
        m = dict(shared)
        m["xT"] = np.ascontiguousarray(xT_all[sl])
        in_maps.append(m)

    ncprog = _get_program()
    res = run_bass_kernel_spmd(
        ncprog, in_maps, list(range(NCORE)), trace=trace,
        tmpdir=os.environ.get("BASS_KERNEL_TRACE_DIR") or None)
    if trace and res.exec_time_ns is not None:
        print(f"HW exec time: {res.exec_time_ns} ns")

    out = np.empty((B, S, D), f32)
    klsum = 0.0
    for c in range(NCORE):
        r = res.results[c]
        oT = r["outT"]
        out[2 * c] = oT[0].T
        out[2 * c + 1] = oT[1].T
        klsum += float(r["klp"][0, 0])
        if c < NCORE - 1:
            klsum += float(r["klp"][0, 1])
    kl = np.float32(klsum / (B - 1))
    return out, kl


# revision 14
# speedup vs baseline: 1.0547x; 1.0547x over previous
"""Trainium2 Bass kernel for nn_EnergyTransformerEncoderLayer_51402168598868.

Strategy: data-parallel over batch B=16 across 8 NeuronCores (2 batches per
core).  Each core also receives batch 2c+2 (wrapped) so the KL term's
(aw[b], self_attn[b+1]) pairs can be computed entirely locally; the host sums
the 15 valid pair contributions.  All activations use "T-space" layouts
(feature dim on partitions) so every matmul contracts over partitions; the
host pre-transposes x and the weights, and transposes the output back.
Matmuls run as float32r (full PE rate, ~1e-4 rel rounding).
"""

import os
import sys
import types

if "/opt/trn_rl_repo" not in sys.path:
    sys.path.insert(0, "/opt/trn_rl_repo")

import numpy as np

B, S, D, F = 16, 1024, 1024, 4096
NCORE = 8
EPS = 1e-6
LN_EPS = 1e-5
P = 128
DC = D // P          # 8 feature chunks
SC = S // P          # 8 sequence chunks
FC = F // P          # 32 ffn chunks
H = 512              # matmul free-dim tile (one PSUM bank of f32)
NH = S // H          # 2 halves

_CACHE = {}


def _install_ntff_hook():
    """Optional: register the NTFF profiling hook (for HW exec timing)."""
    try:
        import antenv
        if getattr(antenv, "axon_hooks", None) is not None:
            return
        hooks = types.ModuleType("antenv.axon_hooks")
        hooks._hook = None

        def set_axon_ntff_profile_hook(h):
            hooks._hook = h

        def get_axon_ntff_profile_hook():
            return hooks._hook

        hooks.set_axon_ntff_profile_hook = set_axon_ntff_profile_hook
        hooks.get_axon_ntff_profile_hook = get_axon_ntff_profile_hook
        antenv.axon_hooks = hooks
        sys.modules["antenv.axon_hooks"] = hooks
        from trn_agent_boot.trn_boot import _ntff_profile_via_ctypes
        set_axon_ntff_profile_hook(
            _ntff_profile_via_ctypes("/opt/axon/libaxon_pjrt.so"))
    except Exception:
        pass


def _build():
    import concourse.mybir as mybir
    import concourse.tile as tile
    from concourse import bacc

    F32 = mybir.dt.float32
    F32R = mybir.dt.float32r
    AF = mybir.ActivationFunctionType
    ALU = mybir.AluOpType
    AX = mybir.AxisListType

    nc = bacc.Bacc(None, target_bir_lowering=False)

    # ---- DRAM parameters (per-core) ----
    xT = nc.declare_dram_parameter("xT", [3, D, S], F32R, isOutput=False)
    WqB = nc.declare_dram_parameter("WqB", [DC, P, DC, P], F32R, isOutput=False)
    WkB = nc.declare_dram_parameter("WkB", [DC, P, DC, P], F32R, isOutput=False)
    WvT = nc.declare_dram_parameter("WvT", [D, D], F32R, isOutput=False)
    WoB = nc.declare_dram_parameter("WoB", [DC, P, DC, P], F32R, isOutput=False)
    W1B = nc.declare_dram_parameter("W1B", [FC, P, DC, P], F32R, isOutput=False)
    W2B = nc.declare_dram_parameter("W2B", [4, DC, P, DC, P], F32R,
                                    isOutput=False)
    bq32 = nc.declare_dram_parameter("bq32", [D], F32, isOutput=False)
    bkv = nc.declare_dram_parameter("bk", [D], F32, isOutput=False)
    boe = nc.declare_dram_parameter("bo_eff", [D], F32, isOutput=False)
    b1v = nc.declare_dram_parameter("b1", [F], F32, isOutput=False)
    b2v = nc.declare_dram_parameter("b2", [D], F32, isOutput=False)
    g1v = nc.declare_dram_parameter("g1", [D], F32, isOutput=False)
    be1v = nc.declare_dram_parameter("be1", [D], F32, isOutput=False)
    g2v = nc.declare_dram_parameter("g2", [D], F32, isOutput=False)
    be2v = nc.declare_dram_parameter("be2", [D], F32, isOutput=False)
    outT = nc.declare_dram_parameter("outT", [2, D, S], F32, isOutput=True)
    klp = nc.declare_dram_parameter("klp", [1, 2], F32, isOutput=True)

    with tile.TileContext(nc) as tc:
        arena = tc.alloc_tile_pool(name="arena", bufs=4)
        bcp = tc.alloc_tile_pool(name="bcp", bufs=2)
        chk = tc.alloc_tile_pool(name="chk", bufs=4)
        wst = tc.alloc_tile_pool(name="wst", bufs=2)
        rows = tc.alloc_tile_pool(name="rows", bufs=3)
        tiny = tc.alloc_tile_pool(name="tiny", bufs=8)
        sing = tc.alloc_tile_pool(name="sing", bufs=1)
        psmm = tc.alloc_tile_pool(name="psmm", bufs=6, space="PSUM")
        psrow = tc.alloc_tile_pool(name="psrow", bufs=2, space="PSUM")
        dram = tc.alloc_tile_pool(name="dram", bufs=1, space="DRAM")

        def big(dtype=F32R, shape=None):
            return arena.tile(shape or [P, DC, S], dtype, tag="big", name="big")

        # ---- constants / persistent small tiles ----
        def vec_pc(name, dram_ap, n_chunks):
            t = sing.tile([P, n_chunks], F32, tag=name, name=name)
            nc.sync.dma_start(out=t, in_=dram_ap.rearrange("(c p) -> p c", p=P))
            return t

        bq_t = vec_pc("bq", bq32[:], DC)
        bk_t = vec_pc("bk", bkv[:], DC)
        bo_t = vec_pc("bo", boe[:], DC)
        b2_t = vec_pc("b2", b2v[:], DC)
        g1_t = vec_pc("g1", g1v[:], DC)
        be1_t = vec_pc("be1", be1v[:], DC)
        g2_t = vec_pc("g2", g2v[:], DC)
        be2_t = vec_pc("be2", be2v[:], DC)
        b1_t = vec_pc("b1", b1v[:], FC)

        eps_t = sing.tile([P, 1], F32, tag="eps")
        nc.vector.memset(eps_t, EPS)
        lneps11 = sing.tile([1, 1], F32, tag="lneps")
        nc.vector.memset(lneps11, LN_EPS)
        ones_f = sing.tile([P, 1], F32, tag="onesf")
        nc.vector.memset(ones_f, 1.0)
        ones_r = sing.tile([P, 1], F32R, tag="onesr")
        nc.vector.tensor_copy(out=ones_r, in_=ones_f)

        nm_t = [sing.tile([P, SC], F32, tag=f"nm{i}", name=f"nm{i}") for i in range(2)]
        rinv_t = [sing.tile([P, SC], F32, tag=f"rinv{i}", name=f"rinv{i}") for i in range(2)]
        kl_cols = sing.tile([P, SC], F32, tag="klcols")
        spill = [dram.tile([P, SC, S], F32, tag=f"sp{i}", name=f"sp{i}") for i in range(2)]

        # ================= helpers =================

        def proj(dst, w_blk, src, bias_pc, scale, evict):
            """dst[o, s] = evict(sum_k W[k, o].T @ src[k, s]); weights
            streamed as [P, DC, P] pre-blocked column blocks."""
            for oc in range(DC):
                wcol = wst.tile([P, DC, P], F32R, tag="wcol", name="wcol")
                nc.sync.dma_start(out=wcol, in_=w_blk[oc])
                for h in range(NH):
                    ps = psmm.tile([P, H], F32, tag="ps", name="ps")
                    for kc in range(DC):
                        nc.tensor.matmul(
                            ps,
                            wcol[:, kc, :],
                            src[:, kc, h * H:(h + 1) * H],
                            start=(kc == 0), stop=(kc == DC - 1))
                    evict(dst, oc, h, ps, bias_pc, scale)

        def evict_act(dst, oc, h, ps, bias_pc, scale):
            nc.scalar.activation(
                out=dst[:, oc, h * H:(h + 1) * H], in_=ps, func=AF.Identity,
                bias=bias_pc[:, oc:oc + 1], scale=scale)

        def evict_copy(dst, oc, h, ps, bias_pc, scale):
            nc.vector.tensor_copy(out=dst[:, oc, h * H:(h + 1) * H], in_=ps)

        def softmax_rows(score_mms, nm_col, rinv_col, aw_chunk):
            """score_mms: emits the two psum halves; computes stats and
            evicts exp(score - max) into aw_chunk [P, S], rowsums -> rinv."""
            ps_a, ps_b = score_mms()
            nma = tiny.tile([P, 1], F32, tag="t", name="t")
            nmb = tiny.tile([P, 1], F32, tag="t", name="t")
            nc.vector.tensor_reduce(out=nma, in_=ps_a, axis=AX.X, op=ALU.max,
                                    negate=True)
            nc.vector.tensor_reduce(out=nmb, in_=ps_b, axis=AX.X, op=ALU.max,
                                    negate=True)
            nc.vector.tensor_tensor(out=nm_col, in0=nma, in1=nmb, op=ALU.min)
            rsa = tiny.tile([P, 1], F32, tag="t", name="t")
            rsb = tiny.tile([P, 1], F32, tag="t", name="t")
            nc.scalar.activation(out=aw_chunk[:, 0:H], in_=ps_a, func=AF.Exp,
                                 bias=nm_col, scale=1.0, accum_out=rsa)
            nc.scalar.activation(out=aw_chunk[:, H:S], in_=ps_b, func=AF.Exp,
                                 bias=nm_col, scale=1.0, accum_out=rsb)
            rs = tiny.tile([P, 1], F32, tag="t", name="t")
            nc.vector.tensor_tensor(out=rs, in0=rsa, in1=rsb, op=ALU.add)
            nc.vector.reciprocal(out=rinv_col, in_=rs)

        def layer_norm_T(x_tile, g_pc, be_pc):
            """In-place LayerNorm over the feature (partition x chunk) dim of
            x_tile [P, DC, S] (f32r):  x <- (x - mu) * rstd * g  (+ be via ACT
            if be_pc given).  2 DVE passes:  t = (x*g)*rstd_b ;
            x = (negmu_rstd_b*g) + t."""
            xf = x_tile.bitcast(F32)
            nmu_row = rows.tile([1, S], F32, tag="row", name="row")
            m2_row = rows.tile([1, S], F32, tag="row", name="row")
            for h in range(NH):
                pr = psrow.tile([1, H], F32, tag="pr", name="pr")
                for c in range(DC):
                    nc.tensor.matmul(pr, ones_r,
                                     x_tile[:, c, h * H:(h + 1) * H],
                                     start=(c == 0), stop=(c == DC - 1))
                nc.scalar.activation(out=nmu_row[0:1, h * H:(h + 1) * H],
                                     in_=pr, func=AF.Copy, scale=-1.0 / D)
                pr2 = psrow.tile([1, H], F32, tag="pr", name="pr")
                for c in range(DC):
                    sq = chk.tile([P, H], F32R, tag="c", name="c")
                    nc.vector.tensor_tensor(
                        out=sq, in0=xf[:, c, h * H:(h + 1) * H],
                        in1=xf[:, c, h * H:(h + 1) * H], op=ALU.mult)
                    nc.tensor.matmul(pr2, ones_r, sq,
                                     start=(c == 0), stop=(c == DC - 1))
                nc.scalar.activation(out=m2_row[0:1, h * H:(h + 1) * H],
                                     in_=pr2, func=AF.Copy, scale=1.0 / D)
            musq = rows.tile([1, S], F32, tag="row", name="row")
            nc.vector.tensor_tensor(out=musq, in0=nmu_row, in1=nmu_row,
                                    op=ALU.mult)
            nc.vector.tensor_tensor(out=m2_row, in0=m2_row, in1=musq,
                                    op=ALU.subtract)
            nc.scalar.activation(out=m2_row, in_=m2_row, func=AF.Sqrt,
                                 bias=lneps11, scale=1.0)
            nc.vector.reciprocal_approx_fast(out=m2_row, in_=m2_row)  # rstd
            nc.vector.tensor_tensor(out=nmu_row, in0=nmu_row, in1=m2_row,
                                    op=ALU.mult)  # -mu * rstd
            nmur_b = bcp.tile([P, S], F32, tag="bc", name="bc")
            rstd_b = bcp.tile([P, S], F32, tag="bc", name="bc")
            nc.gpsimd.partition_broadcast(nmur_b, nmu_row[0:1, :])
            nc.gpsimd.partition_broadcast(rstd_b, m2_row[0:1, :])
            for c in range(DC):
                t = chk.tile([P, S], F32, tag="c", name="c")
                nc.vector.scalar_tensor_tensor(
                    out=t, in0=xf[:, c, :], scalar=g_pc[:, c:c + 1],
                    in1=rstd_b, op0=ALU.mult, op1=ALU.mult)
                nc.vector.scalar_tensor_tensor(
                    out=x_tile[:, c, :], in0=nmur_b,
                    scalar=g_pc[:, c:c + 1], in1=t,
                    op0=ALU.mult, op1=ALU.add)
                if be_pc is not None:
                    nc.scalar.activation(
                        out=x_tile[:, c, :], in_=x_tile[:, c, :],
                        func=AF.Identity, bias=be_pc[:, c:c + 1], scale=1.0)

        def kl_pair(pair_idx, a_slot, xb_tile):
            """KL contribution of (aw[a_slot] from spill, sa from xb_tile)."""
            for sc_i in range(SC):
                def sa_mms(sc_i=sc_i):
                    out = []
                    for h in range(NH):
                        ps = psmm.tile([P, H], F32, tag="ps", name="ps")
                        for kc in range(DC):
                            nc.tensor.matmul(
                                ps,
                                xb_tile[:, kc, sc_i * P:(sc_i + 1) * P],
                                xb_tile[:, kc, h * H:(h + 1) * H],
                                start=(kc == 0), stop=(kc == DC - 1))
                        out.append(ps)
                    return out
                e_sa = chk.tile([P, S], F32, tag="c", name="c")
                nm_sa = tiny.tile([P, 1], F32, tag="t", name="t")
                rinv_sa = tiny.tile([P, 1], F32, tag="t", name="t")
                softmax_rows(sa_mms, nm_sa, rinv_sa, e_sa)
                # rcp = 1 / (sa + EPS), in place over e_sa
                nc.vector.tensor_scalar(out=e_sa, in0=e_sa, scalar1=rinv_sa,
                                        scalar2=EPS, op0=ALU.mult, op1=ALU.add)
                nc.vector.reciprocal_approx_fast(out=e_sa, in_=e_sa)
                aw_c = chk.tile([P, S], F32, tag="c", name="c")
                nc.sync.dma_start(out=aw_c, in_=spill[a_slot][:, sc_i, :])
                klq = chk.tile([P, S], F32, tag="c", name="c")
                nc.vector.scalar_tensor_tensor(
                    out=klq, in0=aw_c, scalar=rinv_t[a_slot][:, sc_i:sc_i + 1],
                    in1=e_sa, op0=ALU.mult, op1=ALU.mult)
                nc.scalar.activation(out=klq, in_=klq, func=AF.Ln,
                                     bias=eps_t, scale=1.0)
                klr = tiny.tile([P, 1], F32, tag="t", name="t")
                nc.vector.scalar_tensor_tensor(
                    out=klq, in0=aw_c, scalar=rinv_t[a_slot][:, sc_i:sc_i + 1],
                    in1=klq, op0=ALU.mult, op1=ALU.mult, accum_out=klr)
                nc.vector.tensor_copy(out=kl_cols[:, sc_i:sc_i + 1], in_=klr)
            # reduce kl_cols over partitions and chunks -> klp[0, pair_idx]
            prk = psrow.tile([1, SC], F32, tag="pr", name="pr")
            nc.tensor.matmul(prk, ones_f.bitcast(F32), kl_cols,
                             start=True, stop=True)
            kl11 = tiny.tile([1, 1], F32, tag="t", name="t")
            nc.vector.tensor_reduce(out=kl11, in_=prk, axis=AX.X, op=ALU.add)
            klsb = tiny.tile([1, 1], F32, tag="t", name="t")
            nc.vector.tensor_copy(out=klsb, in_=kl11)
            nc.sync.dma_start(out=klp[0:1, pair_idx:pair_idx + 1], in_=klsb)

        # ================= main per-slot loop =================
        for slot in range(2):
            # --- load xT for this slot ---
            xTs = big()
            nc.sync.dma_start(
                out=xTs, in_=xT[slot].rearrange("(c p) s -> p c s", p=P))
            xfs = xTs.bitcast(F32)

            # --- Q/K projections (scaled by 1/32 on Q) ---
            qT = big()
            proj(qT, WqB, xTs, bq_t, 1.0 / 32.0, evict_act)
            kT = big()
            proj(kT, WkB, xTs, bk_t, 1.0, evict_act)

            # --- scores softmax (S-layout) -> aw spill + stats ---
            for sc_i in range(SC):
                def sc_mms(sc_i=sc_i):
                    out = []
                    for h in range(NH):
                        ps = psmm.tile([P, H], F32, tag="ps", name="ps")
                        for kc in range(DC):
                            nc.tensor.matmul(
                                ps,
                                qT[:, kc, sc_i * P:(sc_i + 1) * P],
                                kT[:, kc, h * H:(h + 1) * H],
                                start=(kc == 0), stop=(kc == DC - 1))
                        out.append(ps)
                    return out
                aw_c = chk.tile([P, S], F32, tag="c", name="c")
                nm_sc = tiny.tile([P, 1], F32, tag="t", name="t")
                rinv_sc = tiny.tile([P, 1], F32, tag="t", name="t")
                softmax_rows(sc_mms, nm_sc, rinv_sc, aw_c)
                nc.vector.tensor_copy(out=nm_t[slot][:, sc_i:sc_i + 1],
                                      in_=nm_sc)
                nc.vector.tensor_copy(out=rinv_t[slot][:, sc_i:sc_i + 1],
                                      in_=rinv_sc)
                nc.sync.dma_start(out=spill[slot][:, sc_i, :], in_=aw_c)

            # --- awT = exp(scoresT), unnormalized (T-layout) ---
            awT = big()
            for tc_i in range(SC):
                for h in range(NH):
                    ps = psmm.tile([P, H], F32, tag="ps", name="ps")
                    for kc in range(DC):
                        nc.tensor.matmul(
                            ps,
                            kT[:, kc, tc_i * P:(tc_i + 1) * P],
                            qT[:, kc, h * H:(h + 1) * H],
                            start=(kc == 0), stop=(kc == DC - 1))
                    nc.scalar.activation(
                        out=awT[:, tc_i, h * H:(h + 1) * H], in_=ps,
                        func=AF.Exp, scale=1.0)

            # --- combo row: exp(-max)*rinv, broadcast over partitions ---
            combo_pc = tiny.tile([P, SC], F32, tag="combo", name="combo")
            nc.scalar.activation(out=combo_pc, in_=nm_t[slot], func=AF.Exp,
                                 scale=1.0)
            nc.vector.tensor_tensor(out=combo_pc, in0=combo_pc,
                                    in1=rinv_t[slot], op=ALU.mult)
            combo_d = dram.tile([S], F32, tag=f"combod{slot}", name="combod")
            nc.gpsimd.dma_start(
                out=combo_d.rearrange("(c p) -> p c", p=P), in_=combo_pc)
            combo_row = rows.tile([1, S], F32, tag="row", name="row")
            nc.gpsimd.dma_start(out=combo_row, in_=combo_d.rearrange("(o s) -> o s", o=1))
            combo_b = bcp.tile([P, S], F32, tag="bc", name="bc")
            nc.gpsimd.partition_broadcast(combo_b, combo_row[0:1, :])

            # --- V projection (normal [t, i] layout) ---
            # V.T[i, t] would need xT as rhs; instead compute V[t, i] with
            # WvT chunks as rhs, streamed [P, S] per kc (uses each chunk for
            # all tc so keep kc outer over psum pairs per tc: restructure
            # with tc outer and wv chunks cached in wst (8 x 4KB).
            vN = big()
            wv_chunks = []
            for kc in range(DC):
                wvk = wst.tile([P, S], F32R, tag=f"wv{kc}", name="wvk", bufs=1)
                nc.sync.dma_start(
                    out=wvk, in_=WvT[kc * P:(kc + 1) * P, :])
                wv_chunks.append(wvk)
            for tc_i in range(SC):
                for h in range(NH):
                    ps = psmm.tile([P, H], F32, tag="ps", name="ps")
                    for kc in range(DC):
                        nc.tensor.matmul(
                            ps,
                            xTs[:, kc, tc_i * P:(tc_i + 1) * P],
                            wv_chunks[kc][:, h * H:(h + 1) * H],
                            start=(kc == 0), stop=(kc == DC - 1))
                    nc.vector.tensor_copy(
                        out=vN[:, tc_i, h * H:(h + 1) * H], in_=ps)

            # --- attnvT[i, s] = sum_t V[t, i] * awT[t, s], scaled by combo ---
            avT = big()
            for ic in range(DC):
                for h in range(NH):
                    ps = psmm.tile([P, H], F32, tag="ps", name="ps")
                    for tc_i in range(SC):
                        nc.tensor.matmul(
                            ps,
                            vN[:, tc_i, ic * P:(ic + 1) * P],
                            awT[:, tc_i, h * H:(h + 1) * H],
                            start=(tc_i == 0), stop=(tc_i == SC - 1))
                    nc.vector.tensor_tensor(
                        out=avT[:, ic, h * H:(h + 1) * H], in0=ps,
                        in1=combo_b[:, h * H:(h + 1) * H], op=ALU.mult)

            # --- output projection + residual -> hpre (becomes hT) ---
            hT = big()
            for oc in range(DC):
                wocol = wst.tile([P, DC, P], F32R, tag="wcol", name="wcol")
                nc.sync.dma_start(out=wocol, in_=WoB[oc])
                for h in range(NH):
                    ps = psmm.tile([P, H], F32, tag="ps", name="ps")
                    for kc in range(DC):
                        nc.tensor.matmul(
                            ps,
                            wocol[:, kc, :],
                            avT[:, kc, h * H:(h + 1) * H],
                            start=(kc == 0), stop=(kc == DC - 1))
                    nc.vector.scalar_tensor_tensor(
                        out=hT[:, oc, h * H:(h + 1) * H], in0=ps,
                        scalar=bo_t[:, oc:oc + 1],
                        in1=xfs[:, oc, h * H:(h + 1) * H],
                        op0=ALU.add, op1=ALU.add)

            # --- KL pairs (only emitted in slot 1; needs sa of this slot) ---
            if slot == 1:
                kl_pair(0, 0, xTs)

            # --- LN1 in place: hT = LN(x + attn_out) ---
            layer_norm_T(hT, g1_t, None)

            if slot == 1:
                xT2 = big()
                nc.sync.dma_start(
                    out=xT2, in_=xT[2].rearrange("(c p) s -> p c s", p=P))
                kl_pair(1, 1, xT2)

            # --- FFN ---
            opre = big()
            opf = opre.bitcast(F32)
            hf = hT.bitcast(F32)
            for h in range(NH):
                for fh in range(2):
                    gT = big(shape=[P, FC // 2, H])
                    for fl in range(FC // 2):
                        fabs = fh * (FC // 2) + fl
                        w1f = wst.tile([P, DC, P], F32R, tag="wcol", name="w1f")
                        nc.sync.dma_start(out=w1f, in_=W1B[fabs])
                        ps = psmm.tile([P, H], F32, tag="ps", name="ps")
                        for kc in range(DC):
                            nc.tensor.matmul(
                                ps, w1f[:, kc, :],
                                hT[:, kc, h * H:(h + 1) * H],
                                start=(kc == 0), stop=(kc == DC - 1))
                        nc.scalar.activation(
                            out=gT[:, fl, :], in_=ps, func=AF.Relu,
                            bias=b1_t[:, fabs:fabs + 1], scale=1.0)
                    for ic in range(DC):
                        ps = psmm.tile([P, H], F32, tag="ps", name="ps")
                        for fq in range(2):
                            w2q = wst.tile([P, DC, P], F32R, tag="wcol",
                                           name="w2q")
                            nc.sync.dma_start(
                                out=w2q, in_=W2B[fh * 2 + fq, ic])
                            for fl in range(DC):
                                fg = fq * DC + fl
                                nc.tensor.matmul(
                                    ps, w2q[:, fl, :], gT[:, fg, :],
                                    start=(fg == 0),
                                    stop=(fg == FC // 2 - 1))
                        if fh == 0:
                            nc.vector.scalar_tensor_tensor(
                                out=opre[:, ic, h * H:(h + 1) * H], in0=ps,
                                scalar=b2_t[:, ic:ic + 1],
                                in1=hf[:, ic, h * H:(h + 1) * H],
                                op0=ALU.add, op1=ALU.add)
                        else:
                            nc.vector.tensor_tensor(
                                out=opre[:, ic, h * H:(h + 1) * H], in0=ps,
                                in1=opf[:, ic, h * H:(h + 1) * H], op=ALU.add)

            # --- LN2 in place -> final output, DMA out ---
            layer_norm_T(opre, g2_t, be2_t)
            for c in range(DC):
                nc.sync.dma_start(out=outT[slot, c * P:(c + 1) * P, :],
                                  in_=opf[:, c, :])

        for p in (dram, psrow, psmm, sing, tiny, rows, wst, chk, bcp, arena):
            p.release()

    nc.compile()
    return nc


def _get_program():
    if "nc" not in _CACHE:
        _CACHE["nc"] = _build()
    return _CACHE["nc"]


def kernel(x, Wq, bq, Wk, bk, Wv, bv, Wo, bo, g1, be1, W1, b1, W2, b2, g2, be2):
    from concourse.bass_utils import run_bass_kernel_spmd

    trace = os.environ.get("BASS_KERNEL_TRACE", "") == "1"
    if trace:
        _install_ntff_hook()

    f32 = np.float32
    x = np.asarray(x, f32)
    asf = lambda a: np.ascontiguousarray(np.asarray(a, f32))
    def col_blocks(WT):
        # WT [K, O] -> [O/P, P, K/P, P]: blk[oc, p, c, j] = WT[c*P+p, oc*P+j]
        return np.ascontiguousarray(
            WT.reshape(DC, P, -1, P).transpose(2, 1, 0, 3))

    WqBn = col_blocks(np.asarray(Wq, f32).T)
    WkBn = col_blocks(np.asarray(Wk, f32).T)
    WvTn = asf(np.asarray(Wv, f32).T)
    WoBn = col_blocks(np.asarray(Wo, f32).T)
    # W1T [D, F] -> [F/P, P, D/P, P]
    W1Bn = np.ascontiguousarray(
        np.asarray(W1, f32).T.reshape(DC, P, FC, P).transpose(2, 1, 0, 3))
    # W2T [F, D] -> [4, D/P, P, F/(4P), P]: blk[g, ic, p, fl, j]
    W2Bn = np.ascontiguousarray(
        np.asarray(W2, f32).T.reshape(4, DC, P, DC, P).transpose(0, 3, 2, 1, 4))
    bq32n = asf(np.asarray(bq, f32) / 32.0)
    # attnv is computed without +bv; fold the exact linear correction into bo
    bo_eff = asf(np.asarray(bo, f32) + np.asarray(Wo, f32) @ np.asarray(bv, f32))
    xT_all = np.ascontiguousarray(x.transpose(0, 2, 1))

    # LN1's additive bias be1 folds exactly into the FFN biases:
    #   relu(h@W1.T + b1) with h = h' + be1  ->  b1_eff = b1 + W1 @ be1
    #   out-LN input (h + ff)               ->  b2_eff = b2 + be1
    b1_eff = asf(np.asarray(b1, f32) + np.asarray(W1, f32) @ np.asarray(be1, f32))
    b2_eff = asf(np.asarray(b2, f32) + np.asarray(be1, f32))
    shared = dict(
        WqB=WqBn, WkB=WkBn, WvT=WvTn, WoB=WoBn, W1B=W1Bn, W2B=W2Bn,
        bq32=bq32n, bk=asf(bk), bo_eff=bo_eff, b1=b1_eff, b2=b2_eff,
        g1=asf(g1), be1=asf(be1), g2=asf(g2), be2=asf(be2))

    in_maps = []
    for c in range(NCORE):
        sl = [2 * c, 2 * c + 1, (2 * c + 2) % B]
        m = dict(shared)
        m["xT"] = np.ascontiguousarray(xT_all[sl])
        in_maps.append(m)

    ncprog = _get_program()
    res = run_bass_kernel_spmd(
        ncprog, in_maps, list(range(NCORE)), trace=trace,
        tmpdir=os.environ.get("BASS_KERNEL_TRACE_DIR") or None)
    if trace and res.exec_time_ns is not None:
        print(f"HW exec time: {res.exec_time_ns} ns")

    out = np.empty((B, S, D), f32)
    klsum = 0.0
    for c in range(NCORE):
        r = res.results[c]
        oT = r["outT"]
        out[2 * c] = oT[0].T
        out[2 * c + 1] = oT[1].T
        klsum += float(r["klp"][0, 0])
        if c < NCORE - 1:
            klsum += float(r["klp"][0, 1])
    kl = np.float32(klsum / (B - 1))
    return out, kl


# revision 15
# speedup vs baseline: 1.4664x; 1.3904x over previous
"""Trainium2 Bass kernel for nn_EnergyTransformerEncoderLayer_51402168598868.

Strategy: data-parallel over batch B=16 across 8 NeuronCores (2 batches per
core).  Each core also receives batch 2c+2 (wrapped) so the KL term's
(aw[b], self_attn[b+1]) pairs can be computed entirely locally; the host sums
the 15 valid pair contributions.  All activations use "T-space" layouts
(feature dim on partitions) so every matmul contracts over partitions; the
host pre-transposes x and the weights, and transposes the output back.
Matmuls run as float32r (full PE rate, ~1e-4 rel rounding).
"""

import os
import sys
import types

if "/opt/trn_rl_repo" not in sys.path:
    sys.path.insert(0, "/opt/trn_rl_repo")

import numpy as np

B, S, D, F = 16, 1024, 1024, 4096
NCORE = 8
EPS = 1e-6
LN_EPS = 1e-5
P = 128
DC = D // P          # 8 feature chunks
SC = S // P          # 8 sequence chunks
FC = F // P          # 32 ffn chunks
H = 512              # matmul free-dim tile (one PSUM bank of f32)
NH = S // H          # 2 halves

_CACHE = {}


def _install_ntff_hook():
    """Optional: register the NTFF profiling hook (for HW exec timing)."""
    try:
        import antenv
        if getattr(antenv, "axon_hooks", None) is not None:
            return
        hooks = types.ModuleType("antenv.axon_hooks")
        hooks._hook = None

        def set_axon_ntff_profile_hook(h):
            hooks._hook = h

        def get_axon_ntff_profile_hook():
            return hooks._hook

        hooks.set_axon_ntff_profile_hook = set_axon_ntff_profile_hook
        hooks.get_axon_ntff_profile_hook = get_axon_ntff_profile_hook
        antenv.axon_hooks = hooks
        sys.modules["antenv.axon_hooks"] = hooks
        from trn_agent_boot.trn_boot import _ntff_profile_via_ctypes
        set_axon_ntff_profile_hook(
            _ntff_profile_via_ctypes("/opt/axon/libaxon_pjrt.so"))
    except Exception:
        pass


def _build():
    import concourse.mybir as mybir
    import concourse.tile as tile
    from concourse import bacc

    F32 = mybir.dt.float32
    F32R = mybir.dt.float32r
    AF = mybir.ActivationFunctionType
    ALU = mybir.AluOpType
    AX = mybir.AxisListType

    nc = bacc.Bacc(None, target_bir_lowering=False)

    # ---- DRAM parameters (per-core) ----
    xT = nc.declare_dram_parameter("xT", [3, D, S], F32R, isOutput=False)
    WqB = nc.declare_dram_parameter("WqB", [DC, P, DC, P], F32R, isOutput=False)
    WkB = nc.declare_dram_parameter("WkB", [DC, P, DC, P], F32R, isOutput=False)
    WvT = nc.declare_dram_parameter("WvT", [D, D], F32R, isOutput=False)
    WoB = nc.declare_dram_parameter("WoB", [DC, P, DC, P], F32R, isOutput=False)
    W1B = nc.declare_dram_parameter("W1B", [FC, P, DC, P], F32R, isOutput=False)
    W2B = nc.declare_dram_parameter("W2B", [4, DC, P, DC, P], F32R,
                                    isOutput=False)
    bq32 = nc.declare_dram_parameter("bq32", [D], F32, isOutput=False)
    bkv = nc.declare_dram_parameter("bk", [D], F32, isOutput=False)
    boe = nc.declare_dram_parameter("bo_eff", [D], F32, isOutput=False)
    b1v = nc.declare_dram_parameter("b1", [F], F32, isOutput=False)
    b2v = nc.declare_dram_parameter("b2", [D], F32, isOutput=False)
    g1v = nc.declare_dram_parameter("g1", [D], F32, isOutput=False)
    be1v = nc.declare_dram_parameter("be1", [D], F32, isOutput=False)
    g2v = nc.declare_dram_parameter("g2", [D], F32, isOutput=False)
    be2v = nc.declare_dram_parameter("be2", [D], F32, isOutput=False)
    outT = nc.declare_dram_parameter("outT", [2, D, S], F32, isOutput=True)
    klp = nc.declare_dram_parameter("klp", [1, 2], F32, isOutput=True)

    with tile.TileContext(nc) as tc:
        arena = tc.alloc_tile_pool(name="arena", bufs=4)
        bcp = tc.alloc_tile_pool(name="bcp", bufs=2)
        chk = tc.alloc_tile_pool(name="chk", bufs=4)
        wst = tc.alloc_tile_pool(name="wst", bufs=10)
        rows = tc.alloc_tile_pool(name="rows", bufs=3)
        tiny = tc.alloc_tile_pool(name="tiny", bufs=8)
        sing = tc.alloc_tile_pool(name="sing", bufs=1)
        psmm = tc.alloc_tile_pool(name="psmm", bufs=6, space="PSUM")
        psrow = tc.alloc_tile_pool(name="psrow", bufs=2, space="PSUM")
        dram = tc.alloc_tile_pool(name="dram", bufs=1, space="DRAM")

        def big(dtype=F32R, shape=None):
            return arena.tile(shape or [P, DC, S], dtype, tag="big", name="big")

        # ---- constants / persistent small tiles ----
        def vec_pc(name, dram_ap, n_chunks):
            t = sing.tile([P, n_chunks], F32, tag=name, name=name)
            nc.sync.dma_start(out=t, in_=dram_ap.rearrange("(c p) -> p c", p=P))
            return t

        bq_t = vec_pc("bq", bq32[:], DC)
        bk_t = vec_pc("bk", bkv[:], DC)
        bo_t = vec_pc("bo", boe[:], DC)
        b2_t = vec_pc("b2", b2v[:], DC)
        g1_t = vec_pc("g1", g1v[:], DC)
        be1_t = vec_pc("be1", be1v[:], DC)
        g2_t = vec_pc("g2", g2v[:], DC)
        be2_t = vec_pc("be2", be2v[:], DC)
        b1_t = vec_pc("b1", b1v[:], FC)

        eps_t = sing.tile([P, 1], F32, tag="eps")
        nc.vector.memset(eps_t, EPS)
        lneps11 = sing.tile([1, 1], F32, tag="lneps")
        nc.vector.memset(lneps11, LN_EPS)
        ones_f = sing.tile([P, 1], F32, tag="onesf")
        nc.vector.memset(ones_f, 1.0)
        ones_r = sing.tile([P, 1], F32R, tag="onesr")
        nc.vector.tensor_copy(out=ones_r, in_=ones_f)

        nm_t = [sing.tile([P, SC], F32, tag=f"nm{i}", name=f"nm{i}") for i in range(2)]
        rinv_t = [sing.tile([P, SC], F32, tag=f"rinv{i}", name=f"rinv{i}") for i in range(2)]
        kl_cols = sing.tile([P, SC], F32, tag="klcols")
        spill = [dram.tile([P, SC, S], F32, tag=f"sp{i}", name=f"sp{i}") for i in range(2)]

        # ================= helpers =================

        def proj(dst, w_blk, src, bias_pc, scale, evict):
            """dst[o, s] = evict(sum_k W[k, o].T @ src[k, s]); weights
            streamed as [P, DC, P] pre-blocked column blocks."""
            for oc in range(DC):
                wcol = wst.tile([P, DC, P], F32R, tag="wcol", name="wcol")
                nc.sync.dma_start(out=wcol, in_=w_blk[oc])
                for h in range(NH):
                    ps = psmm.tile([P, H], F32, tag="ps", name="ps")
                    for kc in range(DC):
                        nc.tensor.matmul(
                            ps,
                            wcol[:, kc, :],
                            src[:, kc, h * H:(h + 1) * H],
                            start=(kc == 0), stop=(kc == DC - 1))
                    evict(dst, oc, h, ps, bias_pc, scale)

        def evict_act(dst, oc, h, ps, bias_pc, scale):
            nc.scalar.activation(
                out=dst[:, oc, h * H:(h + 1) * H], in_=ps, func=AF.Identity,
                bias=bias_pc[:, oc:oc + 1], scale=scale)

        def evict_copy(dst, oc, h, ps, bias_pc, scale):
            nc.vector.tensor_copy(out=dst[:, oc, h * H:(h + 1) * H], in_=ps)

        def softmax_rows(score_mms, nm_col, rinv_col, aw_chunk):
            """score_mms: emits the two psum halves; computes stats and
            evicts exp(score - max) into aw_chunk [P, S], rowsums -> rinv."""
            ps_a, ps_b = score_mms()
            nma = tiny.tile([P, 1], F32, tag="t", name="t")
            nmb = tiny.tile([P, 1], F32, tag="t", name="t")
            nc.vector.tensor_reduce(out=nma, in_=ps_a, axis=AX.X, op=ALU.max,
                                    negate=True)
            nc.vector.tensor_reduce(out=nmb, in_=ps_b, axis=AX.X, op=ALU.max,
                                    negate=True)
            nc.vector.tensor_tensor(out=nm_col, in0=nma, in1=nmb, op=ALU.min)
            rsa = tiny.tile([P, 1], F32, tag="t", name="t")
            rsb = tiny.tile([P, 1], F32, tag="t", name="t")
            nc.scalar.activation(out=aw_chunk[:, 0:H], in_=ps_a, func=AF.Exp,
                                 bias=nm_col, scale=1.0, accum_out=rsa)
            nc.scalar.activation(out=aw_chunk[:, H:S], in_=ps_b, func=AF.Exp,
                                 bias=nm_col, scale=1.0, accum_out=rsb)
            rs = tiny.tile([P, 1], F32, tag="t", name="t")
            nc.vector.tensor_tensor(out=rs, in0=rsa, in1=rsb, op=ALU.add)
            nc.vector.reciprocal(out=rinv_col, in_=rs)

        def layer_norm_T(x_tile, g_pc, be_pc):
            """In-place LayerNorm over the feature (partition x chunk) dim of
            x_tile [P, DC, S] (f32r):  x <- (x - mu) * rstd * g  (+ be via ACT
            if be_pc given).  2 DVE passes:  t = (x*g)*rstd_b ;
            x = (negmu_rstd_b*g) + t."""
            xf = x_tile.bitcast(F32)
            nmu_row = rows.tile([1, S], F32, tag="row", name="row")
            m2_row = rows.tile([1, S], F32, tag="row", name="row")
            for h in range(NH):
                pr = psrow.tile([1, H], F32, tag="pr", name="pr")
                for c in range(DC):
                    nc.tensor.matmul(pr, ones_r,
                                     x_tile[:, c, h * H:(h + 1) * H],
                                     start=(c == 0), stop=(c == DC - 1))
                nc.scalar.activation(out=nmu_row[0:1, h * H:(h + 1) * H],
                                     in_=pr, func=AF.Copy, scale=-1.0 / D)
                pr2 = psrow.tile([1, H], F32, tag="pr", name="pr")
                for c in range(DC):
                    sq = chk.tile([P, H], F32R, tag="c", name="c")
                    nc.vector.tensor_tensor(
                        out=sq, in0=xf[:, c, h * H:(h + 1) * H],
                        in1=xf[:, c, h * H:(h + 1) * H], op=ALU.mult)
                    nc.tensor.matmul(pr2, ones_r, sq,
                                     start=(c == 0), stop=(c == DC - 1))
                nc.scalar.activation(out=m2_row[0:1, h * H:(h + 1) * H],
                                     in_=pr2, func=AF.Copy, scale=1.0 / D)
            musq = rows.tile([1, S], F32, tag="row", name="row")
            nc.vector.tensor_tensor(out=musq, in0=nmu_row, in1=nmu_row,
                                    op=ALU.mult)
            nc.vector.tensor_tensor(out=m2_row, in0=m2_row, in1=musq,
                                    op=ALU.subtract)
            nc.scalar.activation(out=m2_row, in_=m2_row, func=AF.Sqrt,
                                 bias=lneps11, scale=1.0)
            nc.vector.reciprocal_approx_fast(out=m2_row, in_=m2_row)  # rstd
            nc.vector.tensor_tensor(out=nmu_row, in0=nmu_row, in1=m2_row,
                                    op=ALU.mult)  # -mu * rstd
            nmur_b = bcp.tile([P, S], F32, tag="bc", name="bc")
            rstd_b = bcp.tile([P, S], F32, tag="bc", name="bc")
            nc.gpsimd.partition_broadcast(nmur_b, nmu_row[0:1, :])
            nc.gpsimd.partition_broadcast(rstd_b, m2_row[0:1, :])
            for c in range(DC):
                t = chk.tile([P, S], F32, tag="c", name="c")
                nc.vector.scalar_tensor_tensor(
                    out=t, in0=xf[:, c, :], scalar=g_pc[:, c:c + 1],
                    in1=rstd_b, op0=ALU.mult, op1=ALU.mult)
                nc.vector.scalar_tensor_tensor(
                    out=x_tile[:, c, :], in0=nmur_b,
                    scalar=g_pc[:, c:c + 1], in1=t,
                    op0=ALU.mult, op1=ALU.add)
                if be_pc is not None:
                    nc.scalar.activation(
                        out=x_tile[:, c, :], in_=x_tile[:, c, :],
                        func=AF.Identity, bias=be_pc[:, c:c + 1], scale=1.0)

        def kl_pair(pair_idx, a_slot, xb_tile):
            """KL contribution of (aw[a_slot] from spill, sa from xb_tile)."""
            for sc_i in range(SC):
                def sa_mms(sc_i=sc_i):
                    out = []
                    for h in range(NH):
                        ps = psmm.tile([P, H], F32, tag="ps", name="ps")
                        for kc in range(DC):
                            nc.tensor.matmul(
                                ps,
                                xb_tile[:, kc, sc_i * P:(sc_i + 1) * P],
                                xb_tile[:, kc, h * H:(h + 1) * H],
                                start=(kc == 0), stop=(kc == DC - 1))
                        out.append(ps)
                    return out
                e_sa = chk.tile([P, S], F32, tag="c", name="c")
                nm_sa = tiny.tile([P, 1], F32, tag="t", name="t")
                rinv_sa = tiny.tile([P, 1], F32, tag="t", name="t")
                softmax_rows(sa_mms, nm_sa, rinv_sa, e_sa)
                # rcp = 1 / (sa + EPS), in place over e_sa
                nc.vector.tensor_scalar(out=e_sa, in0=e_sa, scalar1=rinv_sa,
                                        scalar2=EPS, op0=ALU.mult, op1=ALU.add)
                nc.vector.reciprocal_approx_fast(out=e_sa, in_=e_sa)
                aw_c = chk.tile([P, S], F32, tag="c", name="c")
                nc.sync.dma_start(out=aw_c, in_=spill[a_slot][:, sc_i, :])
                klq = chk.tile([P, S], F32, tag="c", name="c")
                nc.vector.scalar_tensor_tensor(
                    out=klq, in0=aw_c, scalar=rinv_t[a_slot][:, sc_i:sc_i + 1],
                    in1=e_sa, op0=ALU.mult, op1=ALU.mult)
                nc.scalar.activation(out=klq, in_=klq, func=AF.Ln,
                                     bias=eps_t, scale=1.0)
                klr = tiny.tile([P, 1], F32, tag="t", name="t")
                nc.vector.scalar_tensor_tensor(
                    out=klq, in0=aw_c, scalar=rinv_t[a_slot][:, sc_i:sc_i + 1],
                    in1=klq, op0=ALU.mult, op1=ALU.mult, accum_out=klr)
                nc.vector.tensor_copy(out=kl_cols[:, sc_i:sc_i + 1], in_=klr)
            # reduce kl_cols over partitions and chunks -> klp[0, pair_idx]
            prk = psrow.tile([1, SC], F32, tag="pr", name="pr")
            nc.tensor.matmul(prk, ones_f.bitcast(F32), kl_cols,
                             start=True, stop=True)
            kl11 = tiny.tile([1, 1], F32, tag="t", name="t")
            nc.vector.tensor_reduce(out=kl11, in_=prk, axis=AX.X, op=ALU.add)
            klsb = tiny.tile([1, 1], F32, tag="t", name="t")
            nc.vector.tensor_copy(out=klsb, in_=kl11)
            nc.sync.dma_start(out=klp[0:1, pair_idx:pair_idx + 1], in_=klsb)

        # ================= main per-slot loop =================
        xT_next = None
        for slot in range(2):
            # --- load xT for this slot (slot 1 was prefetched in slot 0) ---
            if xT_next is None:
                xTs = big()
                nc.sync.dma_start(
                    out=xTs, in_=xT[slot].rearrange("(c p) s -> p c s", p=P))
            else:
                xTs = xT_next
            xfs = xTs.bitcast(F32)

            # --- Q/K projections (scaled by 1/32 on Q) ---
            qT = big()
            proj(qT, WqB, xTs, bq_t, 1.0 / 32.0, evict_act)
            kT = big()
            proj(kT, WkB, xTs, bk_t, 1.0, evict_act)

            # --- scores softmax (S-layout) -> aw spill + stats ---
            for sc_i in range(SC):
                def sc_mms(sc_i=sc_i):
                    out = []
                    for h in range(NH):
                        ps = psmm.tile([P, H], F32, tag="ps", name="ps")
                        for kc in range(DC):
                            nc.tensor.matmul(
                                ps,
                                qT[:, kc, sc_i * P:(sc_i + 1) * P],
                                kT[:, kc, h * H:(h + 1) * H],
                                start=(kc == 0), stop=(kc == DC - 1))
                        out.append(ps)
                    return out
                aw_c = chk.tile([P, S], F32, tag="c", name="c")
                nm_sc = tiny.tile([P, 1], F32, tag="t", name="t")
                rinv_sc = tiny.tile([P, 1], F32, tag="t", name="t")
                softmax_rows(sc_mms, nm_sc, rinv_sc, aw_c)
                nc.vector.tensor_copy(out=nm_t[slot][:, sc_i:sc_i + 1],
                                      in_=nm_sc)
                nc.vector.tensor_copy(out=rinv_t[slot][:, sc_i:sc_i + 1],
                                      in_=rinv_sc)
                nc.sync.dma_start(out=spill[slot][:, sc_i, :], in_=aw_c)

            # --- awT = exp(scoresT), unnormalized (T-layout) ---
            awT = big()
            for tc_i in range(SC):
                for h in range(NH):
                    ps = psmm.tile([P, H], F32, tag="ps", name="ps")
                    for kc in range(DC):
                        nc.tensor.matmul(
                            ps,
                            kT[:, kc, tc_i * P:(tc_i + 1) * P],
                            qT[:, kc, h * H:(h + 1) * H],
                            start=(kc == 0), stop=(kc == DC - 1))
                    nc.scalar.activation(
                        out=awT[:, tc_i, h * H:(h + 1) * H], in_=ps,
                        func=AF.Exp, scale=1.0)

            # --- combo row: exp(-max)*rinv, broadcast over partitions ---
            combo_pc = tiny.tile([P, SC], F32, tag="combo", name="combo")
            nc.scalar.activation(out=combo_pc, in_=nm_t[slot], func=AF.Exp,
                                 scale=1.0)
            nc.vector.tensor_tensor(out=combo_pc, in0=combo_pc,
                                    in1=rinv_t[slot], op=ALU.mult)
            combo_d = dram.tile([S], F32, tag=f"combod{slot}", name="combod")
            nc.gpsimd.dma_start(
                out=combo_d.rearrange("(c p) -> p c", p=P), in_=combo_pc)
            combo_row = rows.tile([1, S], F32, tag="row", name="row")
            nc.gpsimd.dma_start(out=combo_row, in_=combo_d.rearrange("(o s) -> o s", o=1))
            combo_b = bcp.tile([P, S], F32, tag="bc", name="bc")
            nc.gpsimd.partition_broadcast(combo_b, combo_row[0:1, :])

            # --- V projection (normal [t, i] layout) ---
            # V.T[i, t] would need xT as rhs; instead compute V[t, i] with
            # WvT chunks as rhs, streamed [P, S] per kc (uses each chunk for
            # all tc so keep kc outer over psum pairs per tc: restructure
            # with tc outer and wv chunks cached in wst (8 x 4KB).
            vN = big()
            wv_chunks = []
            for kc in range(DC):
                wvk = wst.tile([P, S], F32R, tag="wcol", name="wvk")
                nc.sync.dma_start(
                    out=wvk, in_=WvT[kc * P:(kc + 1) * P, :])
                wv_chunks.append(wvk)
            for tc_i in range(SC):
                for h in range(NH):
                    ps = psmm.tile([P, H], F32, tag="ps", name="ps")
                    for kc in range(DC):
                        nc.tensor.matmul(
                            ps,
                            xTs[:, kc, tc_i * P:(tc_i + 1) * P],
                            wv_chunks[kc][:, h * H:(h + 1) * H],
                            start=(kc == 0), stop=(kc == DC - 1))
                    nc.vector.tensor_copy(
                        out=vN[:, tc_i, h * H:(h + 1) * H], in_=ps)

            # --- attnvT[i, s] = sum_t V[t, i] * awT[t, s], scaled by combo ---
            avT = big()
            for ic in range(DC):
                for h in range(NH):
                    ps = psmm.tile([P, H], F32, tag="ps", name="ps")
                    for tc_i in range(SC):
                        nc.tensor.matmul(
                            ps,
                            vN[:, tc_i, ic * P:(ic + 1) * P],
                            awT[:, tc_i, h * H:(h + 1) * H],
                            start=(tc_i == 0), stop=(tc_i == SC - 1))
                    nc.vector.tensor_tensor(
                        out=avT[:, ic, h * H:(h + 1) * H], in0=ps,
                        in1=combo_b[:, h * H:(h + 1) * H], op=ALU.mult)

            # --- output projection + residual -> hpre (becomes hT) ---
            hT = big()
            for oc in range(DC):
                wocol = wst.tile([P, DC, P], F32R, tag="wcol", name="wcol")
                nc.sync.dma_start(out=wocol, in_=WoB[oc])
                for h in range(NH):
                    ps = psmm.tile([P, H], F32, tag="ps", name="ps")
                    for kc in range(DC):
                        nc.tensor.matmul(
                            ps,
                            wocol[:, kc, :],
                            avT[:, kc, h * H:(h + 1) * H],
                            start=(kc == 0), stop=(kc == DC - 1))
                    nc.vector.scalar_tensor_tensor(
                        out=hT[:, oc, h * H:(h + 1) * H], in0=ps,
                        scalar=bo_t[:, oc:oc + 1],
                        in1=xfs[:, oc, h * H:(h + 1) * H],
                        op0=ALU.add, op1=ALU.add)

            # --- LN1 in place: hT = LN(x + attn_out) ---
            layer_norm_T(hT, g1_t, None)

            if slot == 1:
                xT2 = big()
                nc.sync.dma_start(
                    out=xT2, in_=xT[2].rearrange("(c p) s -> p c s", p=P))
                kl_pair(1, 1, xT2)

            # --- FFN ---
            opre = big()
            opf = opre.bitcast(F32)
            hf = hT.bitcast(F32)
            for h in range(NH):
                for fh in range(2):
                    gT = big(shape=[P, FC // 2, H])
                    for fl in range(FC // 2):
                        fabs = fh * (FC // 2) + fl
                        w1f = wst.tile([P, DC, P], F32R, tag="wcol", name="w1f")
                        nc.sync.dma_start(out=w1f, in_=W1B[fabs])
                        ps = psmm.tile([P, H], F32, tag="ps", name="ps")
                        for kc in range(DC):
                            nc.tensor.matmul(
                                ps, w1f[:, kc, :],
                                hT[:, kc, h * H:(h + 1) * H],
                                start=(kc == 0), stop=(kc == DC - 1))
                        nc.scalar.activation(
                            out=gT[:, fl, :], in_=ps, func=AF.Relu,
                            bias=b1_t[:, fabs:fabs + 1], scale=1.0)
                    for ic in range(DC):
                        ps = psmm.tile([P, H], F32, tag="ps", name="ps")
                        for fq in range(2):
                            w2q = wst.tile([P, DC, P], F32R, tag="wcol",
                                           name="w2q")
                            nc.sync.dma_start(
                                out=w2q, in_=W2B[fh * 2 + fq, ic])
                            for fl in range(DC):
                                fg = fq * DC + fl
                                nc.tensor.matmul(
                                    ps, w2q[:, fl, :], gT[:, fg, :],
                                    start=(fg == 0),
                                    stop=(fg == FC // 2 - 1))
                        if fh == 0:
                            nc.vector.scalar_tensor_tensor(
                                out=opre[:, ic, h * H:(h + 1) * H], in0=ps,
                                scalar=b2_t[:, ic:ic + 1],
                                in1=hf[:, ic, h * H:(h + 1) * H],
                                op0=ALU.add, op1=ALU.add)
                        else:
                            nc.vector.tensor_tensor(
                                out=opre[:, ic, h * H:(h + 1) * H], in0=ps,
                                in1=opf[:, ic, h * H:(h + 1) * H], op=ALU.add)

            # --- end of slot 0: prefetch x[1], run KL pair 0 (aw0 vs sa(x1)).
            # Its matmuls and vector chains fill the LN2 / slot-boundary gaps.
            if slot == 0:
                xT_next = big()
                nc.sync.dma_start(
                    out=xT_next,
                    in_=xT[1].rearrange("(c p) s -> p c s", p=P))
                kl_pair(0, 0, xT_next)

            # --- LN2 in place -> final output, DMA out ---
            layer_norm_T(opre, g2_t, be2_t)
            for c in range(DC):
                nc.sync.dma_start(out=outT[slot, c * P:(c + 1) * P, :],
                                  in_=opf[:, c, :])

        for p in (dram, psrow, psmm, sing, tiny, rows, wst, chk, bcp, arena):
            p.release()

    nc.compile()
    return nc


def _get_program():
    if "nc" not in _CACHE:
        _CACHE["nc"] = _build()
    return _CACHE["nc"]


def kernel(x, Wq, bq, Wk, bk, Wv, bv, Wo, bo, g1, be1, W1, b1, W2, b2, g2, be2):
    from concourse.bass_utils import run_bass_kernel_spmd

    trace = os.environ.get("BASS_KERNEL_TRACE", "") == "1"
    if trace:
        _install_ntff_hook()

    f32 = np.float32
    x = np.asarray(x, f32)
    asf = lambda a: np.ascontiguousarray(np.asarray(a, f32))
    def col_blocks(WT):
        # WT [K, O] -> [O/P, P, K/P, P]: blk[oc, p, c, j] = WT[c*P+p, oc*P+j]
        return np.ascontiguousarray(
            WT.reshape(DC, P, -1, P).transpose(2, 1, 0, 3))

    WqBn = col_blocks(np.asarray(Wq, f32).T)
    WkBn = col_blocks(np.asarray(Wk, f32).T)
    WvTn = asf(np.asarray(Wv, f32).T)
    WoBn = col_blocks(np.asarray(Wo, f32).T)
    # W1T [D, F] -> [F/P, P, D/P, P]
    W1Bn = np.ascontiguousarray(
        np.asarray(W1, f32).T.reshape(DC, P, FC, P).transpose(2, 1, 0, 3))
    # W2T [F, D] -> [4, D/P, P, F/(4P), P]: blk[g, ic, p, fl, j]
    W2Bn = np.ascontiguousarray(
        np.asarray(W2, f32).T.reshape(4, DC, P, DC, P).transpose(0, 3, 2, 1, 4))
    bq32n = asf(np.asarray(bq, f32) / 32.0)
    # attnv is computed without +bv; fold the exact linear correction into bo
    bo_eff = asf(np.asarray(bo, f32) + np.asarray(Wo, f32) @ np.asarray(bv, f32))
    xT_all = np.ascontiguousarray(x.transpose(0, 2, 1))

    # LN1's additive bias be1 folds exactly into the FFN biases:
    #   relu(h@W1.T + b1) with h = h' + be1  ->  b1_eff = b1 + W1 @ be1
    #   out-LN input (h + ff)               ->  b2_eff = b2 + be1
    b1_eff = asf(np.asarray(b1, f32) + np.asarray(W1, f32) @ np.asarray(be1, f32))
    b2_eff = asf(np.asarray(b2, f32) + np.asarray(be1, f32))
    shared = dict(
        WqB=WqBn, WkB=WkBn, WvT=WvTn, WoB=WoBn, W1B=W1Bn, W2B=W2Bn,
        bq32=bq32n, bk=asf(bk), bo_eff=bo_eff, b1=b1_eff, b2=b2_eff,
        g1=asf(g1), be1=asf(be1), g2=asf(g2), be2=asf(be2))

    in_maps = []
    for c in range(NCORE):
        sl = [2 * c, 2 * c + 1, (2 * c + 2) % B]
        m = dict(shared)
        m["xT"] = np.ascontiguousarray(xT_all[sl])
        in_maps.append(m)

    ncprog = _get_program()
    res = run_bass_kernel_spmd(
        ncprog, in_maps, list(range(NCORE)), trace=trace,
        tmpdir=os.environ.get("BASS_KERNEL_TRACE_DIR") or None)
    if trace and res.exec_time_ns is not None:
        print(f"HW exec time: {res.exec_time_ns} ns")

    out = np.empty((B, S, D), f32)
    klsum = 0.0
    for c in range(NCORE):
        r = res.results[c]
        oT = r["outT"]
        out[2 * c] = oT[0].T
        out[2 * c + 1] = oT[1].T
        klsum += float(r["klp"][0, 0])
        if c < NCORE - 1:
            klsum += float(r["klp"][0, 1])
    kl = np.float32(klsum / (B - 1))
    return out, kl
